# revision 42
# baseline (speedup 1.0000x reference)
"""Trainium2 Bass kernel for the binary-conv BasicBlock (dense_cnn).

Computation (forward values only):
  A1   = sign(x + b11)
  out1 = x + bn1(conv3x3(A1, binw(w3)))          binw(w) = mean|w| * sign(w)
  o1   = prelu(out1 + b12, a1) + b13
  A2   = sign(o1 + b21)
  out2 = bn2(conv1x1(A2, binw(w1))) + o1
  out  = prelu(out2 + b22, a2) + b23

Strategy: data-parallel over batch, 4 images/core on 8 cores.  Per core:
  - A1 shipped from host as H1 = (x+b11 >= 0) in {0,1} fp8 (padded+halo
    layout); conv(2H-1) border correction M1 is folded into xprep on host.
  - xprep = x + (ch1+b12) - sh1*M1 shipped as bf16 (halves input DMA).
  - conv3x3 = 9 shifted fp8 DoubleRow matmuls over row-aligned 464-wide
    tiles (8 rows x 58); conv1x1 = one DR matmul per tile.
  - DVE stt: t1 = psum1*(2sh1) + xprep   (bf16 out)
  - ACT Prelu: p1 = prelu(t1, a1)        (full strips)
  - A2 binarization split across engines with per-half weight scaling:
      kc0: DVE is_ge -> {0,1}, conv2 weights for this half are +-2
      kc1: ACT Sign  -> +-1,  conv2 weights for this half are +-1
    (thr = prelu^-1(-(b13+b21)); requires a1 >= 0, else numpy fallback;
     the {0,1} half's rowsum correction is folded into K2')
  - conv2 residual, split for engine balance (Pool cannot read PSUM):
      mc0: DVE stt t2 = psum2*(2sh2) + p1; ACT Prelu strip -> out
      mc1: PE identity matmul psum2 += diag(1/(2sh2)) @ p1 (bf16), then
           ACT Prelu reads PSUM directly (scale=2sh2, bias=K2') -> out
  - host adds b23 and converts to fp32 during unshard.
  conv2 of image i-1 is interleaved into conv1 of image i so the PE never
  idles (keeps it at the 2.4GHz pstate).
"""

import numpy as np
import ml_dtypes

C = 256
H = W = 56
PH = 58                    # padded image side
NPIX = PH * PH             # 3364
IP = 56 * PH               # 3248: interior rows (1..57) x full width
OP = H * W                 # 3136 compact output pixels
HOFF = 16                  # halo before H1 data (16B aligned block)
H1BLK = 3392               # 16 + 3364 + 12, multiple of 16
NT = 7                     # row tiles per image
TN = 8 * PH                # 464 columns per tile (8 rows)
BPC = 4                    # images per core
NCORES = 8
EPS = 1e-5

_CACHE = {}


def _split_drain_waits(m, max_waits=1):
    """This toolchain's walrus rejects instructions carrying more than ~1-2
    sync waits; hoist extra waits onto preceding single-wait EventSemaphore
    ops on the same engine (semantically identical: the engine blocks on
    each wait in sequence before executing the instruction)."""
    import copy as _copy
    from concourse import mybir

    new_module = _copy.replace(m, functions=[])
    for function in m.functions:
        new_function = _copy.replace(function, blocks=[])
        new_function.set_allocations_from_list(function.allocations)
        for block in function.blocks:
            out = []
            for inst in block.instructions:
                si = inst.sync_info
                if si is not None and len(si.on_wait) > max_waits:
                    waits = list(si.on_wait)
                    keep = waits[:max_waits] if not isinstance(
                        inst, mybir.InstDrain) else []
                    hoist = waits[len(keep):]
                    for i, wt in enumerate(hoist):
                        out.append(
                            mybir.InstEventSemaphore(
                                name=f"{inst.name}-wsplit{i}",
                                opcode="EventSemaphore",
                                engine=inst.engine,
                                sync_info=mybir.SyncInfo(on_wait=[wt], on_update=[]),
                            )
                        )
                    inst.sync_info = mybir.SyncInfo(
                        on_wait=keep, on_update=list(si.on_update)
                    )
                out.append(inst)
            new_block = _copy.replace(block, instructions=out)
            new_function.blocks.append(new_block)
        new_module.functions.append(new_function)
    return new_module


def build_nc():
    """Build (once) the per-core Bass program."""
    if "nc" in _CACHE:
        return _CACHE["nc"]
    import concourse.bass as bass
    import concourse.tile as tile
    from concourse import mybir

    Alu = mybir.AluOpType
    AF = mybir.ActivationFunctionType
    f32 = mybir.dt.float32
    bf16 = mybir.dt.bfloat16
    fp8 = mybir.dt.float8e4
    DR = mybir.MatmulPerfMode.DoubleRow

    nc = bass.Bass(trn_type="TRN2", debug=False)
    x_d = nc.dram_tensor("xprep", [BPC, 2, 128, NPIX], bf16, kind="ExternalInput")
    h_d = nc.dram_tensor("h1", [BPC, 128, 2 * H1BLK], fp8, kind="ExternalInput")
    w3_d = nc.dram_tensor("w3f", [128, 9 * 2 * 2 * 128], fp8, kind="ExternalInput")
    w1_d = nc.dram_tensor("w1f", [128, 2 * 2 * 128], fp8, kind="ExternalInput")
    c_d = nc.dram_tensor("consts", [2, 128, 8], f32, kind="ExternalInput")
    o_d = nc.dram_tensor("out", [BPC, 2, 128, OP], bf16, kind="ExternalOutput")

    with tile.TileContext(nc) as tc:
        with (
            tc.tile_pool(name="wpool", bufs=1) as wpool,
            tc.tile_pool(name="xpool", bufs=2) as xpool,
            tc.tile_pool(name="hpool", bufs=2) as hpool,
            tc.tile_pool(name="t1pool", bufs=2) as t1pool,
            tc.tile_pool(name="p1pool", bufs=2) as p1pool,
            tc.tile_pool(name="a2pool", bufs=2) as a2pool,
            tc.tile_pool(name="t2pool", bufs=2) as t2pool,
            tc.tile_pool(name="opool", bufs=2) as opool,
            tc.tile_pool(name="ps1", bufs=2, space="PSUM") as ps1p,
            tc.tile_pool(name="ps2", bufs=2, space="PSUM") as ps2p,
        ):
            # ---- constants / weights (resident) ----
            w3sb = wpool.tile([128, 9 * 2 * 2 * 128], fp8, tag="w3")
            nc.sync.dma_start(w3sb[:], w3_d.ap())
            w1sb = wpool.tile([128, 2 * 2 * 128], fp8, tag="w1")
            nc.sync.dma_start(w1sb[:], w1_d.ap())
            # [p, (sh mc), 2, m] / [p, mc, 2, m] views for DoubleRow lhsT
            w3v = w3sb[:].rearrange("p (g two m) -> p g two m", two=2, m=128)
            w1v = w1sb[:].rearrange("p (g two m) -> p g two m", two=2, m=128)
            csb = []
            for kc in range(2):
                ct = wpool.tile([128, 8], f32, tag=f"c_{kc}")
                nc.sync.dma_start(ct[:], c_d.ap()[kc])
                csb.append(ct)

            def cc(kc, j):
                # slots: 0=2sh1 1=thr 2=K2' 3=a1 4=a2 5=sh2 6=-thr
                return csb[kc][:, j : j + 1]

            xts = [None] * BPC
            hts = [None] * BPC
            t1ts = [None] * BPC
            p1ts = [None] * BPC
            a2ts = [None] * BPC
            t2ts = [None] * BPC

            def load(img):
                # inputs issued from the (otherwise idle) Pool queue so the
                # Sync queue's ~0.6us-per-issue cost never gates compute.
                # h1 first: the matmuls need it before xprep is touched.
                # Image 0 is chunked so the first slots unblock early;
                # later images load a whole image ahead of use.
                ht = hpool.tile([128, 2 * H1BLK], fp8, tag="h", name=f"h{img}")
                if img == 0:
                    hcut = HOFF + 2064
                    for b0, b1 in ((0, hcut), (hcut, H1BLK)):
                        for kc in range(2):
                            nc.gpsimd.dma_start(
                                ht[:, kc * H1BLK + b0 : kc * H1BLK + b1],
                                h_d.ap()[img][:, kc * H1BLK + b0 :
                                              kc * H1BLK + b1],
                            )
                else:
                    nc.gpsimd.dma_start(ht[:], h_d.ap()[img])
                xt = xpool.tile([128, 2 * NPIX], bf16, tag="x", name=f"x{img}")
                bounds = (
                    [0, PH + 2 * TN, PH + 4 * TN, PH + 6 * TN, NPIX]
                    if img == 0 else [0, NPIX]
                )
                for kc in range(2):
                    for b0, b1 in zip(bounds, bounds[1:]):
                        nc.gpsimd.dma_start(
                            xt[:, kc * NPIX + b0 : kc * NPIX + b1],
                            x_d.ap()[img, kc][:, b0:b1],
                        )
                xts[img], hts[img] = xt, ht

            def conv1_pair(img, tp, chunked=False):
                # tiles 2tp(,2tp+1) share one 2-bank PSUM tile per mc so the
                # DVE drains/binarizes 928 columns per instruction
                if tp == 0:
                    t1ts[img] = t1pool.tile(
                        [128, 2 * IP], bf16, tag="t1", name=f"t1_{img}"
                    )
                    a2ts[img] = a2pool.tile(
                        [128, 2 * IP], fp8, tag="a2", name=f"a2_{img}"
                    )
                t1t, a2t = t1ts[img], a2ts[img]
                hv = hts[img][:].rearrange("p (two w) -> p two w", two=2)
                tiles = [2 * tp] if 2 * tp + 1 >= NT else [2 * tp, 2 * tp + 1]
                c0 = TN * 2 * tp
                n = TN * len(tiles)
                # when chunked (last image), do mc1 first: its ACT Sign
                # gates the same-image interleaved conv2
                for mc in ((1, 0) if chunked else (0, 1)):
                    ps = ps1p.tile([128, 1024], f32, tag="ps1")
                    for j, t in enumerate(tiles):
                        for sh in range(9):
                            kh, kw = divmod(sh, 3)
                            off = HOFF + (8 * t + kh) * PH + (kw - 1)
                            nc.tensor.matmul(
                                ps[:, 512 * j : 512 * j + TN],
                                w3v[:, sh * 2 + mc],
                                hv[:, :, off : off + TN],
                                start=(sh == 0),
                                stop=(sh == 8),
                                perf_mode=DR,
                            )
                    # t1 = psum*(2*sh1) + xprep (x + bn1 + b12 - sh1*M1 folded)
                    if len(tiles) == 2:
                        psv = ps[:].rearrange(
                            "p (two w) -> p two w", two=2
                        )[:, :, :TN]
                        t1v = t1t[:, mc * IP + c0 : mc * IP + c0 + n].rearrange(
                            "p (two w) -> p two w", two=2
                        )
                        xv = xts[img][
                            :, mc * NPIX + PH + c0 : mc * NPIX + PH + c0 + n
                        ].rearrange("p (two w) -> p two w", two=2)
                        nc.vector.scalar_tensor_tensor(
                            t1v, psv, cc(mc, 0), xv, Alu.mult, Alu.add
                        )
                    else:
                        nc.vector.scalar_tensor_tensor(
                            t1t[:, mc * IP + c0 : mc * IP + c0 + n],
                            ps[:, :TN], cc(mc, 0),
                            xts[img][:, mc * NPIX + PH + c0 :
                                      mc * NPIX + PH + c0 + n],
                            Alu.mult, Alu.add,
                        )
                    if mc == 0:
                        # A2 kc0: {0,1} on DVE (conv2 weights +-2 this half)
                        nc.vector.tensor_scalar(
                            a2t[:, c0 : c0 + n], t1t[:, c0 : c0 + n],
                            cc(0, 1), None, Alu.is_ge,
                        )
                    elif chunked:
                        # A2 kc1: +-1 via ACT Sign (bias = -thr), per chunk
                        nc.scalar.activation(
                            a2t[:, IP + c0 : IP + c0 + n],
                            t1t[:, IP + c0 : IP + c0 + n],
                            AF.Sign, bias=cc(1, 6),
                        )
                if tp == 0:
                    p1ts[img] = p1pool.tile(
                        [128, 2 * IP], bf16, tag="p1", name=f"p1_{img}"
                    )
                if chunked:
                    # p1 = prelu(t1, a1) per chunk (ACT)
                    for mc in range(2):
                        nc.scalar.activation(
                            p1ts[img][:, mc * IP + c0 : mc * IP + c0 + n],
                            t1t[:, mc * IP + c0 : mc * IP + c0 + n],
                            AF.Prelu, alpha=cc(mc, 3),
                        )

            def strip1(img):
                # Sign + prelu1 as whole strips: cheaper per element, and
                # their latency is hidden (consumed one image later)
                t1t = t1ts[img]
                nc.scalar.activation(
                    a2ts[img][:, IP : 2 * IP], t1t[:, IP : 2 * IP],
                    AF.Sign, bias=cc(1, 6),
                )
                for mc in range(2):
                    nc.scalar.activation(
                        p1ts[img][:, mc * IP : (mc + 1) * IP],
                        t1t[:, mc * IP : (mc + 1) * IP],
                        AF.Prelu, alpha=cc(mc, 3),
                    )

            ots = [None] * BPC

            def conv2_pair(img, tp):
                if tp == 0:
                    t2ts[img] = t2pool.tile(
                        [128, 2 * IP], bf16, tag="t2", name=f"t2_{img}"
                    )
                    ots[img] = opool.tile(
                        [128, 2 * OP], bf16, tag="o", name=f"o_{img}"
                    )
                t2t = t2ts[img]
                a2v = a2ts[img][:].rearrange("p (two w) -> p two w", two=2)
                tiles = [2 * tp] if 2 * tp + 1 >= NT else [2 * tp, 2 * tp + 1]
                c0 = TN * 2 * tp
                n = TN * len(tiles)
                for mc in range(2):
                    ps = ps2p.tile([128, 1024], f32, tag="ps2")
                    for j, t in enumerate(tiles):
                        nc.tensor.matmul(
                            ps[:, 512 * j : 512 * j + TN],
                            w1v[:, mc],
                            a2v[:, :, TN * t : TN * t + TN],
                            start=True,
                            stop=True,
                            perf_mode=DR,
                        )
                    # t2 = psum*sh2 + p1 on DVE; prelu2 chunk follows (ACT)
                    if len(tiles) == 2:
                        psv = ps[:].rearrange(
                            "p (two w) -> p two w", two=2
                        )[:, :, :TN]
                        t2v = t2t[:, mc * IP + c0 : mc * IP + c0 + n].rearrange(
                            "p (two w) -> p two w", two=2
                        )
                        pv = p1ts[img][
                            :, mc * IP + c0 : mc * IP + c0 + n
                        ].rearrange("p (two w) -> p two w", two=2)
                        nc.vector.scalar_tensor_tensor(
                            t2v, psv, cc(mc, 5), pv, Alu.mult, Alu.add
                        )
                    else:
                        nc.vector.scalar_tensor_tensor(
                            t2t[:, mc * IP + c0 : mc * IP + c0 + n],
                            ps[:, :TN], cc(mc, 5),
                            p1ts[img][:, mc * IP + c0 : mc * IP + c0 + n],
                            Alu.mult, Alu.add,
                        )
                    # out = prelu(t2 + K2', a2), compacted; b23 on host
                    rows = 8 * len(tiles)
                    t2i = t2t[:, mc * IP + c0 : mc * IP + c0 + n].rearrange(
                        "p (h w) -> p h w", h=rows
                    )[:, :, 1:57]
                    oc = 56 * rows
                    o0 = mc * OP + 56 * c0 // PH
                    ov = ots[img][:, o0 : o0 + oc].rearrange(
                        "p (h w) -> p h w", h=rows
                    )
                    nc.scalar.activation(
                        ov, t2i, AF.Prelu, bias=cc(mc, 2), alpha=cc(mc, 4)
                    )

            def strip2(img):
                # chunked out-DMA: first half can fire while the second
                # half's prelu2 chunks are still draining
                ot = ots[img]
                for b0, b1 in ((0, 1792), (1792, OP)):
                    for mc in range(2):
                        nc.sync.dma_start(
                            o_d.ap()[img, mc][:, b0:b1],
                            ot[:, mc * OP + b0 : mc * OP + b1],
                        )

            load(0)
            NP_ = (NT + 1) // 2  # 4 pair-slots
            for img in range(BPC):
                if img + 1 < BPC:
                    load(img + 1)
                last = img == BPC - 1
                for tp in range(NP_):
                    conv1_pair(img, tp, chunked=last)
                    if img >= 1:
                        # prev image's 1x1 conv: inputs long since ready
                        conv2_pair(img - 1, tp)
                    if last and tp >= 2:
                        # last image has no successor to host its conv2;
                        # interleave same-image two slots behind
                        conv2_pair(img, tp - 2)
                if not last:
                    strip1(img)
                if img >= 1:
                    strip2(img - 1)
            for tp in (2, 3):
                conv2_pair(BPC - 1, tp)
            strip2(BPC - 1)

    _CACHE["nc"] = nc
    return nc


def _host_fold(w3, w1, b11, b12, b13, b21, b22, b23,
               g1, be1, m1, v1, g2, be2, m2, v2, a1, a2):
    f = np.float32
    s3 = np.mean(np.abs(w3), axis=(1, 2, 3)).astype(f)
    s1 = np.mean(np.abs(w1), axis=(1, 2, 3)).astype(f)
    inv1 = (g1 / np.sqrt(v1 + EPS)).astype(f)
    inv2 = (g2 / np.sqrt(v2 + EPS)).astype(f)
    sh1 = s3 * inv1
    ch1 = be1 - m1 * inv1
    sh2 = s1 * inv2
    ch2 = be2 - m2 * inv2
    K1 = (ch1 + b12).astype(f)

    sgn3 = np.sign(w3).astype(f)                     # [O, I, 3, 3]
    sgn1 = np.sign(w1).astype(f)
    # M1[c, i, j] = sum over in-bounds taps of rowsum3[c, kh, kw]
    rowsum3 = sgn3.sum(axis=1)                       # [C, 3, 3]
    M1 = np.zeros((C, H, W), f)
    for kh in range(3):
        for kw in range(3):
            ind = np.zeros((H, W), f)
            r0, r1 = max(0, 1 - kh), min(H - 1, H - kh) + 1
            c0, c1 = max(0, 1 - kw), min(W - 1, W - kw) + 1
            ind[r0:r1, c0:c1] = 1.0
            M1 += rowsum3[:, kh, kw][:, None, None] * ind[None]
    xadj = K1[:, None, None] - sh1[:, None, None] * M1   # [C, H, W]

    # A2 threshold: p1 >= -(b13+b21)  <=>  t1 >= thr (prelu inverse, a1>=0)
    u = (-(b13 + b21)).astype(f)
    safe_a1 = np.where(a1 > 0, a1, 1.0).astype(f)
    thr = np.where(u > 0, u, np.where(a1 > 0, u / safe_a1, f(-3e38)))
    # {0,1}-encoded kc0 half: conv(2H-1) correction = rowsum over kc0 inputs
    r1h = sgn1[:, :128].sum(axis=(1, 2, 3)).astype(f)
    K2p = (ch2 + b13 + b22 - sh2 * r1h).astype(f)

    fp8 = ml_dtypes.float8_e4m3
    # DoubleRow lhsT layout: [k, ((sh*2+mc)*2+i)*128+m] with i the K-half
    W3 = sgn3.astype(fp8)                                       # [O, I, 3, 3]
    W3 = W3.reshape(2, 128, 2, 128, 3, 3)                       # [mc,m,i,k,kh,kw]
    W3 = W3.transpose(3, 4, 5, 0, 2, 1)                         # [k,kh,kw,mc,i,m]
    W3f = np.ascontiguousarray(W3.reshape(128, 9 * 2 * 2 * 128))
    W1 = sgn1.reshape(2, 128, 2, 128).copy()                    # [mc, m, i, k]
    W1[:, :, 0, :] *= 2.0              # kc0 half is {0,1}-encoded: +-2
    W1 = W1.astype(fp8).transpose(3, 0, 2, 1)                   # [k, mc, i, m]
    W1f = np.ascontiguousarray(W1.reshape(128, 2 * 2 * 128))

    consts = np.zeros((2, 128, 8), f)
    for kc in range(2):
        sl = slice(kc * 128, (kc + 1) * 128)
        consts[kc, :, 0] = 2.0 * sh1[sl]
        consts[kc, :, 1] = thr[sl]
        consts[kc, :, 2] = K2p[sl]
        consts[kc, :, 3] = a1[sl]
        consts[kc, :, 4] = a2[sl]
        consts[kc, :, 5] = sh2[sl]
        consts[kc, :, 6] = -thr[sl]
    return W3f, W1f, consts, xadj


def _run(in_maps, trace=False, tmpdir=None, trace_kwargs={}):
    from concourse import bass_utils

    nc = build_nc()
    if not _CACHE.get("split"):
        # walrus workaround applied only for the HW path (CoreSim rejects
        # post-scheduling instruction edits)
        nc.m = _split_drain_waits(nc.m)
        _CACHE["split"] = True
    return bass_utils.run_bass_kernel_spmd(
        nc,
        in_maps,
        core_ids=list(range(NCORES)),
        trace=trace,
        tmpdir=tmpdir,
        trace_kwargs=trace_kwargs,
    )


def make_in_maps(x, w3, w1, **params):
    x = np.asarray(x, np.float32)
    params = {k: np.asarray(v, np.float32) for k, v in params.items()}
    W3f, W1f, consts, xadj = _host_fold(np.asarray(w3, np.float32),
                                        np.asarray(w1, np.float32), **params)
    _CACHE["b23"] = params["b23"]
    bf16 = ml_dtypes.bfloat16
    fp8 = ml_dtypes.float8_e4m3
    N = x.shape[0]

    xp = np.zeros((N, C, PH, PH), bf16)
    xp[:, :, 1:57, 1:57] = (x + xadj[None]).astype(bf16)
    x_prep = xp.reshape(N, 2, 128, NPIX)

    hp = np.zeros((N, C, PH, PH), fp8)
    hp[:, :, 1:57, 1:57] = (
        x + params["b11"][None, :, None, None] >= 0
    ).astype(fp8)
    harr = np.zeros((N, 128, 2, H1BLK), fp8)
    harr[:, :, :, HOFF : HOFF + NPIX] = hp.reshape(
        N, 2, 128, NPIX
    ).transpose(0, 2, 1, 3)
    harr = harr.reshape(N, 128, 2 * H1BLK)

    return [
        {
            "xprep": np.ascontiguousarray(x_prep[c * BPC : (c + 1) * BPC]),
            "h1": np.ascontiguousarray(harr[c * BPC : (c + 1) * BPC]),
            "w3f": W3f, "w1f": W1f, "consts": consts,
        }
        for c in range(NCORES)
    ]


def finish_out(arr):
    """Per-core raw out [BPC,2,128,OP] bf16 -> [BPC,C,H,W] fp32 (+b23)."""
    out = np.asarray(arr).reshape(BPC, C, H, W).astype(np.float32)
    return out + _CACHE["b23"][None, :, None, None]


def assemble_out(results):
    outs = [finish_out(results[c]["out"]) for c in range(NCORES)]
    return np.ascontiguousarray(np.concatenate(outs, axis=0))


def _fallback_numpy(x, w3, w1, b11, b12, b13, b21, b22, b23,
                    g1, be1, m1, v1, g2, be2, m2, v2, a1, a2):
    # Straightforward reference math in numpy; only used if an assumption of
    # the device kernel (prelu slope a1 >= 0) is violated.
    def cb(p):
        return p[None, :, None, None]

    def conv_np(a, w, pad):
        N, Ci, Hh, Ww = a.shape
        O, I, kh, kw = w.shape
        ap = np.pad(a, ((0, 0), (0, 0), (pad, pad), (pad, pad)))
        out = np.zeros((N, O, Hh, Ww), np.float32)
        wm = w.reshape(O, -1)
        for n in range(N):
            cols = np.empty((I * kh * kw, Hh * Ww), np.float32)
            idx = 0
            for i in range(I):
                for dh in range(kh):
                    for dw in range(kw):
                        cols[idx] = ap[n, i, dh : dh + Hh, dw : dw + Ww].ravel()
                        idx += 1
            out[n] = (wm @ cols).reshape(O, Hh, Ww)
        return out

    def bn(t, g, b, mm, v):
        inv = g / np.sqrt(v + EPS)
        return t * cb(inv) + cb(b - mm * inv)

    def prelu(t, a):
        return np.where(t > 0, t, cb(a) * t)

    s3 = np.mean(np.abs(w3), axis=(1, 2, 3), keepdims=True)
    s1 = np.mean(np.abs(w1), axis=(1, 2, 3), keepdims=True)
    o1 = conv_np(np.sign(x + cb(b11)), np.sign(w3) * s3, 1)
    o1 = x + bn(o1, g1, be1, m1, v1)
    o1 = prelu(o1 + cb(b12), a1) + cb(b13)
    o2 = conv_np(np.sign(o1 + cb(b21)), np.sign(w1) * s1, 0)
    o2 = bn(o2, g2, be2, m2, v2) + o1
    o2 = prelu(o2 + cb(b22), a2) + cb(b23)
    return o2.astype(np.float32)


def kernel(**inputs):
    inputs = {k: np.asarray(v) for k, v in inputs.items()}
    if (np.asarray(inputs["a1"], np.float32) < 0).any():
        return _fallback_numpy(**{k: np.asarray(v, np.float32)
                                  for k, v in inputs.items()})
    in_maps = make_in_maps(**inputs)
    res = _run(in_maps, trace=False)
    return assemble_out(res.results)


# revision 44
# speedup vs baseline: 1.0628x; 1.0628x over previous
"""Trainium2 Bass kernel for the binary-conv BasicBlock (dense_cnn).

Computation (forward values only):
  A1   = sign(x + b11)
  out1 = x + bn1(conv3x3(A1, binw(w3)))          binw(w) = mean|w| * sign(w)
  o1   = prelu(out1 + b12, a1) + b13
  A2   = sign(o1 + b21)
  out2 = bn2(conv1x1(A2, binw(w1))) + o1
  out  = prelu(out2 + b22, a2) + b23

Strategy: data-parallel over batch, 4 images/core on 8 cores.  Per core:
  - A1 shipped from host as H1 = (x+b11 >= 0) in {0,1} fp8 (padded+halo
    layout); conv(2H-1) border correction M1 is folded into xprep on host.
  - xprep = x + (ch1+b12) - sh1*M1 shipped as bf16 (halves input DMA).
  - conv3x3 = 9 shifted fp8 DoubleRow matmuls over row-aligned 464-wide
    tiles (8 rows x 58); conv1x1 = one DR matmul per tile.
  - DVE stt: t1 = psum1*(2sh1) + xprep   (bf16 out)
  - ACT Prelu: p1 = prelu(t1, a1)        (full strips)
  - A2 binarization split across engines with per-half weight scaling:
      kc0: DVE is_ge -> {0,1}, conv2 weights for this half are +-2
      kc1: ACT Sign  -> +-1,  conv2 weights for this half are +-1
    (thr = prelu^-1(-(b13+b21)); requires a1 >= 0, else numpy fallback;
     the {0,1} half's rowsum correction is folded into K2')
  - conv2 residual, split for engine balance (Pool cannot read PSUM):
      mc0: DVE stt t2 = psum2*(2sh2) + p1; ACT Prelu strip -> out
      mc1: PE identity matmul psum2 += diag(1/(2sh2)) @ p1 (bf16), then
           ACT Prelu reads PSUM directly (scale=2sh2, bias=K2') -> out
  - host adds b23 and converts to fp32 during unshard.
  conv2 of image i-1 is interleaved into conv1 of image i so the PE never
  idles (keeps it at the 2.4GHz pstate).
"""

import numpy as np
import ml_dtypes

C = 256
H = W = 56
PH = 58                    # padded image side
NPIX = PH * PH             # 3364
IP = 56 * PH               # 3248: interior rows (1..57) x full width
OP = H * W                 # 3136 compact output pixels
HOFF = 16                  # halo before H1 data (16B aligned block)
H1BLK = 3392               # 16 + 3364 + 12, multiple of 16
NT = 7                     # row tiles per image
TN = 8 * PH                # 464 columns per tile (8 rows)
BPC = 4                    # images per core
NCORES = 8
EPS = 1e-5

_CACHE = {}


def _split_drain_waits(m, max_waits=1):
    """This toolchain's walrus rejects instructions carrying more than ~1-2
    sync waits; hoist extra waits onto preceding single-wait EventSemaphore
    ops on the same engine (semantically identical: the engine blocks on
    each wait in sequence before executing the instruction)."""
    import copy as _copy
    from concourse import mybir

    new_module = _copy.replace(m, functions=[])
    for function in m.functions:
        new_function = _copy.replace(function, blocks=[])
        new_function.set_allocations_from_list(function.allocations)
        for block in function.blocks:
            out = []
            for inst in block.instructions:
                si = inst.sync_info
                if si is not None and len(si.on_wait) > max_waits:
                    waits = list(si.on_wait)
                    keep = waits[:max_waits] if not isinstance(
                        inst, mybir.InstDrain) else []
                    hoist = waits[len(keep):]
                    for i, wt in enumerate(hoist):
                        out.append(
                            mybir.InstEventSemaphore(
                                name=f"{inst.name}-wsplit{i}",
                                opcode="EventSemaphore",
                                engine=inst.engine,
                                sync_info=mybir.SyncInfo(on_wait=[wt], on_update=[]),
                            )
                        )
                    inst.sync_info = mybir.SyncInfo(
                        on_wait=keep, on_update=list(si.on_update)
                    )
                out.append(inst)
            new_block = _copy.replace(block, instructions=out)
            new_function.blocks.append(new_block)
        new_module.functions.append(new_function)
    return new_module


def build_nc():
    """Build (once) the per-core Bass program."""
    if "nc" in _CACHE:
        return _CACHE["nc"]
    import concourse.bass as bass
    import concourse.tile as tile
    from concourse import mybir

    Alu = mybir.AluOpType
    AF = mybir.ActivationFunctionType
    f32 = mybir.dt.float32
    bf16 = mybir.dt.bfloat16
    fp8 = mybir.dt.float8e4
    DR = mybir.MatmulPerfMode.DoubleRow

    nc = bass.Bass(trn_type="TRN2", debug=False)
    x_d = nc.dram_tensor("xprep", [BPC, 2, 128, NPIX], bf16, kind="ExternalInput")
    h_d = nc.dram_tensor("h1", [BPC, 128, 2 * H1BLK], fp8, kind="ExternalInput")
    w3_d = nc.dram_tensor("w3f", [128, 9 * 2 * 2 * 128], fp8, kind="ExternalInput")
    w1_d = nc.dram_tensor("w1f", [128, 2 * 2 * 128], fp8, kind="ExternalInput")
    c_d = nc.dram_tensor("consts", [2, 128, 8], f32, kind="ExternalInput")
    o_d = nc.dram_tensor("out", [BPC, 2, 128, OP], bf16, kind="ExternalOutput")

    with tile.TileContext(nc) as tc:
        with (
            tc.tile_pool(name="wpool", bufs=1) as wpool,
            tc.tile_pool(name="xpool", bufs=2) as xpool,
            tc.tile_pool(name="hpool", bufs=2) as hpool,
            tc.tile_pool(name="t1pool", bufs=2) as t1pool,
            tc.tile_pool(name="p1pool", bufs=2) as p1pool,
            tc.tile_pool(name="a2pool", bufs=2) as a2pool,
            tc.tile_pool(name="t2pool", bufs=2) as t2pool,
            tc.tile_pool(name="opool", bufs=2) as opool,
            tc.tile_pool(name="ps1", bufs=2, space="PSUM") as ps1p,
            tc.tile_pool(name="ps2", bufs=2, space="PSUM") as ps2p,
        ):
            # ---- constants / weights (resident) ----
            w3sb = wpool.tile([128, 9 * 2 * 2 * 128], fp8, tag="w3")
            nc.sync.dma_start(w3sb[:], w3_d.ap())
            w1sb = wpool.tile([128, 2 * 2 * 128], fp8, tag="w1")
            nc.sync.dma_start(w1sb[:], w1_d.ap())
            # [p, (sh mc), 2, m] / [p, mc, 2, m] views for DoubleRow lhsT
            w3v = w3sb[:].rearrange("p (g two m) -> p g two m", two=2, m=128)
            w1v = w1sb[:].rearrange("p (g two m) -> p g two m", two=2, m=128)
            csb = []
            for kc in range(2):
                ct = wpool.tile([128, 8], f32, tag=f"c_{kc}")
                nc.sync.dma_start(ct[:], c_d.ap()[kc])
                csb.append(ct)

            def cc(kc, j):
                # slots: 0=2sh1 1=thr 2=K2' 3=a1 4=a2 5=sh2 6=-thr
                return csb[kc][:, j : j + 1]

            xts = [None] * BPC
            hts = [None] * BPC
            t1ts = [None] * BPC
            p1ts = [None] * BPC
            a2ts = [None] * BPC
            t2ts = [None] * BPC

            def load(img):
                # inputs issued from the (otherwise idle) Pool queue so the
                # Sync queue's ~0.6us-per-issue cost never gates compute.
                # h1 first: the matmuls need it before xprep is touched.
                # Image 0 is chunked so the first slots unblock early;
                # later images load a whole image ahead of use.
                ht = hpool.tile([128, 2 * H1BLK], fp8, tag="h", name=f"h{img}")
                if img == 0:
                    hcut = HOFF + 2064
                    for b0, b1 in ((0, hcut), (hcut, H1BLK)):
                        for kc in range(2):
                            nc.gpsimd.dma_start(
                                ht[:, kc * H1BLK + b0 : kc * H1BLK + b1],
                                h_d.ap()[img][:, kc * H1BLK + b0 :
                                              kc * H1BLK + b1],
                            )
                else:
                    nc.gpsimd.dma_start(ht[:], h_d.ap()[img])
                xt = xpool.tile([128, 2 * NPIX], bf16, tag="x", name=f"x{img}")
                bounds = (
                    [0, PH + 2 * TN, PH + 4 * TN, PH + 6 * TN, NPIX]
                    if img == 0 else [0, NPIX]
                )
                for kc in range(2):
                    for b0, b1 in zip(bounds, bounds[1:]):
                        nc.gpsimd.dma_start(
                            xt[:, kc * NPIX + b0 : kc * NPIX + b1],
                            x_d.ap()[img, kc][:, b0:b1],
                        )
                xts[img], hts[img] = xt, ht

            def conv1_pair(img, tp, chunked=False):
                # tiles 2tp(,2tp+1) share one 2-bank PSUM tile per mc so the
                # DVE drains/binarizes 928 columns per instruction
                if tp == 0:
                    t1ts[img] = t1pool.tile(
                        [128, 2 * IP], bf16, tag="t1", name=f"t1_{img}"
                    )
                    a2ts[img] = a2pool.tile(
                        [128, 2 * IP], fp8, tag="a2", name=f"a2_{img}"
                    )
                t1t, a2t = t1ts[img], a2ts[img]
                hv = hts[img][:].rearrange("p (two w) -> p two w", two=2)
                tiles = [2 * tp] if 2 * tp + 1 >= NT else [2 * tp, 2 * tp + 1]
                c0 = TN * 2 * tp
                n = TN * len(tiles)
                # when chunked (last image), do mc1 first: its ACT Sign
                # gates the same-image interleaved conv2
                for mc in ((1, 0) if chunked else (0, 1)):
                    ps = ps1p.tile([128, 1024], f32, tag="ps1")
                    for j, t in enumerate(tiles):
                        for sh in range(9):
                            kh, kw = divmod(sh, 3)
                            off = HOFF + (8 * t + kh) * PH + (kw - 1)
                            nc.tensor.matmul(
                                ps[:, 512 * j : 512 * j + TN],
                                w3v[:, sh * 2 + mc],
                                hv[:, :, off : off + TN],
                                start=(sh == 0),
                                stop=(sh == 8),
                                perf_mode=DR,
                            )
                    # t1 = psum*(2*sh1) + xprep (x + bn1 + b12 - sh1*M1 folded)
                    if len(tiles) == 2:
                        psv = ps[:].rearrange(
                            "p (two w) -> p two w", two=2
                        )[:, :, :TN]
                        t1v = t1t[:, mc * IP + c0 : mc * IP + c0 + n].rearrange(
                            "p (two w) -> p two w", two=2
                        )
                        xv = xts[img][
                            :, mc * NPIX + PH + c0 : mc * NPIX + PH + c0 + n
                        ].rearrange("p (two w) -> p two w", two=2)
                        nc.vector.scalar_tensor_tensor(
                            t1v, psv, cc(mc, 0), xv, Alu.mult, Alu.add
                        )
                    else:
                        nc.vector.scalar_tensor_tensor(
                            t1t[:, mc * IP + c0 : mc * IP + c0 + n],
                            ps[:, :TN], cc(mc, 0),
                            xts[img][:, mc * NPIX + PH + c0 :
                                      mc * NPIX + PH + c0 + n],
                            Alu.mult, Alu.add,
                        )
                    if mc == 0:
                        # A2 kc0: {0,1} on DVE (conv2 weights +-2 this half)
                        nc.vector.tensor_scalar(
                            a2t[:, c0 : c0 + n], t1t[:, c0 : c0 + n],
                            cc(0, 1), None, Alu.is_ge,
                        )
                    elif chunked:
                        # A2 kc1: +-1 via ACT Sign (bias = -thr), per chunk
                        nc.scalar.activation(
                            a2t[:, IP + c0 : IP + c0 + n],
                            t1t[:, IP + c0 : IP + c0 + n],
                            AF.Sign, bias=cc(1, 6),
                        )
                if tp == 0:
                    p1ts[img] = p1pool.tile(
                        [128, 2 * IP], bf16, tag="p1", name=f"p1_{img}"
                    )
                if chunked:
                    # p1 = prelu(t1, a1) per chunk (ACT)
                    for mc in range(2):
                        nc.scalar.activation(
                            p1ts[img][:, mc * IP + c0 : mc * IP + c0 + n],
                            t1t[:, mc * IP + c0 : mc * IP + c0 + n],
                            AF.Prelu, alpha=cc(mc, 3),
                        )

            def strip1(img):
                # Sign + prelu1 as whole strips: cheaper per element, and
                # their latency is hidden (consumed one image later)
                t1t = t1ts[img]
                nc.scalar.activation(
                    a2ts[img][:, IP : 2 * IP], t1t[:, IP : 2 * IP],
                    AF.Sign, bias=cc(1, 6),
                )
                for mc in range(2):
                    nc.scalar.activation(
                        p1ts[img][:, mc * IP : (mc + 1) * IP],
                        t1t[:, mc * IP : (mc + 1) * IP],
                        AF.Prelu, alpha=cc(mc, 3),
                    )

            ots = [None] * BPC

            def conv2_pair(img, tp):
                if tp == 0:
                    t2ts[img] = t2pool.tile(
                        [128, 2 * IP], bf16, tag="t2", name=f"t2_{img}"
                    )
                    ots[img] = opool.tile(
                        [128, 2 * OP], bf16, tag="o", name=f"o_{img}"
                    )
                t2t = t2ts[img]
                a2v = a2ts[img][:].rearrange("p (two w) -> p two w", two=2)
                tiles = [2 * tp] if 2 * tp + 1 >= NT else [2 * tp, 2 * tp + 1]
                c0 = TN * 2 * tp
                n = TN * len(tiles)
                for mc in range(2):
                    ps = ps2p.tile([128, 1024], f32, tag="ps2")
                    for j, t in enumerate(tiles):
                        nc.tensor.matmul(
                            ps[:, 512 * j : 512 * j + TN],
                            w1v[:, mc],
                            a2v[:, :, TN * t : TN * t + TN],
                            start=True,
                            stop=True,
                            perf_mode=DR,
                        )
                    # t2 = psum*sh2 + p1 on DVE; prelu2 chunk follows (ACT)
                    if len(tiles) == 2:
                        psv = ps[:].rearrange(
                            "p (two w) -> p two w", two=2
                        )[:, :, :TN]
                        t2v = t2t[:, mc * IP + c0 : mc * IP + c0 + n].rearrange(
                            "p (two w) -> p two w", two=2
                        )
                        pv = p1ts[img][
                            :, mc * IP + c0 : mc * IP + c0 + n
                        ].rearrange("p (two w) -> p two w", two=2)
                        nc.vector.scalar_tensor_tensor(
                            t2v, psv, cc(mc, 5), pv, Alu.mult, Alu.add
                        )
                    else:
                        nc.vector.scalar_tensor_tensor(
                            t2t[:, mc * IP + c0 : mc * IP + c0 + n],
                            ps[:, :TN], cc(mc, 5),
                            p1ts[img][:, mc * IP + c0 : mc * IP + c0 + n],
                            Alu.mult, Alu.add,
                        )
                    # out = prelu(t2 + K2', a2), compacted; b23 on host
                    rows = 8 * len(tiles)
                    t2i = t2t[:, mc * IP + c0 : mc * IP + c0 + n].rearrange(
                        "p (h w) -> p h w", h=rows
                    )[:, :, 1:57]
                    oc = 56 * rows
                    o0 = mc * OP + 56 * c0 // PH
                    ov = ots[img][:, o0 : o0 + oc].rearrange(
                        "p (h w) -> p h w", h=rows
                    )
                    nc.scalar.activation(
                        ov, t2i, AF.Prelu, bias=cc(mc, 2), alpha=cc(mc, 4)
                    )

            def strip2(img):
                # chunked out-DMA: first half can fire while the second
                # half's prelu2 chunks are still draining
                ot = ots[img]
                for b0, b1 in ((0, 1792), (1792, OP)):
                    for mc in range(2):
                        nc.sync.dma_start(
                            o_d.ap()[img, mc][:, b0:b1],
                            ot[:, mc * OP + b0 : mc * OP + b1],
                        )

            load(0)
            NP_ = (NT + 1) // 2  # 4 pair-slots
            for img in range(BPC):
                if img + 1 < BPC:
                    load(img + 1)
                last = img == BPC - 1
                for tp in range(NP_):
                    conv1_pair(img, tp, chunked=True)
                    if img >= 1:
                        # prev image's 1x1 conv: inputs long since ready
                        conv2_pair(img - 1, tp)
                    if last and tp >= 2:
                        # last image has no successor to host its conv2;
                        # interleave same-image two slots behind
                        conv2_pair(img, tp - 2)
                if img >= 1:
                    strip2(img - 1)
            for tp in (2, 3):
                conv2_pair(BPC - 1, tp)
            strip2(BPC - 1)

    _CACHE["nc"] = nc
    return nc


def _host_fold(w3, w1, b11, b12, b13, b21, b22, b23,
               g1, be1, m1, v1, g2, be2, m2, v2, a1, a2):
    f = np.float32
    s3 = np.mean(np.abs(w3), axis=(1, 2, 3)).astype(f)
    s1 = np.mean(np.abs(w1), axis=(1, 2, 3)).astype(f)
    inv1 = (g1 / np.sqrt(v1 + EPS)).astype(f)
    inv2 = (g2 / np.sqrt(v2 + EPS)).astype(f)
    sh1 = s3 * inv1
    ch1 = be1 - m1 * inv1
    sh2 = s1 * inv2
    ch2 = be2 - m2 * inv2
    K1 = (ch1 + b12).astype(f)

    sgn3 = np.sign(w3).astype(f)                     # [O, I, 3, 3]
    sgn1 = np.sign(w1).astype(f)
    # M1[c, i, j] = sum over in-bounds taps of rowsum3[c, kh, kw]
    rowsum3 = sgn3.sum(axis=1)                       # [C, 3, 3]
    M1 = np.zeros((C, H, W), f)
    for kh in range(3):
        for kw in range(3):
            ind = np.zeros((H, W), f)
            r0, r1 = max(0, 1 - kh), min(H - 1, H - kh) + 1
            c0, c1 = max(0, 1 - kw), min(W - 1, W - kw) + 1
            ind[r0:r1, c0:c1] = 1.0
            M1 += rowsum3[:, kh, kw][:, None, None] * ind[None]
    xadj = K1[:, None, None] - sh1[:, None, None] * M1   # [C, H, W]

    # A2 threshold: p1 >= -(b13+b21)  <=>  t1 >= thr (prelu inverse, a1>=0)
    u = (-(b13 + b21)).astype(f)
    safe_a1 = np.where(a1 > 0, a1, 1.0).astype(f)
    thr = np.where(u > 0, u, np.where(a1 > 0, u / safe_a1, f(-3e38)))
    # {0,1}-encoded kc0 half: conv(2H-1) correction = rowsum over kc0 inputs
    r1h = sgn1[:, :128].sum(axis=(1, 2, 3)).astype(f)
    K2p = (ch2 + b13 + b22 - sh2 * r1h).astype(f)

    fp8 = ml_dtypes.float8_e4m3
    # DoubleRow lhsT layout: [k, ((sh*2+mc)*2+i)*128+m] with i the K-half
    W3 = sgn3.astype(fp8)                                       # [O, I, 3, 3]
    W3 = W3.reshape(2, 128, 2, 128, 3, 3)                       # [mc,m,i,k,kh,kw]
    W3 = W3.transpose(3, 4, 5, 0, 2, 1)                         # [k,kh,kw,mc,i,m]
    W3f = np.ascontiguousarray(W3.reshape(128, 9 * 2 * 2 * 128))
    W1 = sgn1.reshape(2, 128, 2, 128).copy()                    # [mc, m, i, k]
    W1[:, :, 0, :] *= 2.0              # kc0 half is {0,1}-encoded: +-2
    W1 = W1.astype(fp8).transpose(3, 0, 2, 1)                   # [k, mc, i, m]
    W1f = np.ascontiguousarray(W1.reshape(128, 2 * 2 * 128))

    consts = np.zeros((2, 128, 8), f)
    for kc in range(2):
        sl = slice(kc * 128, (kc + 1) * 128)
        consts[kc, :, 0] = 2.0 * sh1[sl]
        consts[kc, :, 1] = thr[sl]
        consts[kc, :, 2] = K2p[sl]
        consts[kc, :, 3] = a1[sl]
        consts[kc, :, 4] = a2[sl]
        consts[kc, :, 5] = sh2[sl]
        consts[kc, :, 6] = -thr[sl]
    return W3f, W1f, consts, xadj


def _run(in_maps, trace=False, tmpdir=None, trace_kwargs={}):
    from concourse import bass_utils

    nc = build_nc()
    if not _CACHE.get("split"):
        # walrus workaround applied only for the HW path (CoreSim rejects
        # post-scheduling instruction edits)
        nc.m = _split_drain_waits(nc.m)
        _CACHE["split"] = True
    return bass_utils.run_bass_kernel_spmd(
        nc,
        in_maps,
        core_ids=list(range(NCORES)),
        trace=trace,
        tmpdir=tmpdir,
        trace_kwargs=trace_kwargs,
    )


def make_in_maps(x, w3, w1, **params):
    x = np.asarray(x, np.float32)
    params = {k: np.asarray(v, np.float32) for k, v in params.items()}
    W3f, W1f, consts, xadj = _host_fold(np.asarray(w3, np.float32),
                                        np.asarray(w1, np.float32), **params)
    _CACHE["b23"] = params["b23"]
    bf16 = ml_dtypes.bfloat16
    fp8 = ml_dtypes.float8_e4m3
    N = x.shape[0]

    xp = np.zeros((N, C, PH, PH), bf16)
    xp[:, :, 1:57, 1:57] = (x + xadj[None]).astype(bf16)
    x_prep = xp.reshape(N, 2, 128, NPIX)

    hp = np.zeros((N, C, PH, PH), fp8)
    hp[:, :, 1:57, 1:57] = (
        x + params["b11"][None, :, None, None] >= 0
    ).astype(fp8)
    harr = np.zeros((N, 128, 2, H1BLK), fp8)
    harr[:, :, :, HOFF : HOFF + NPIX] = hp.reshape(
        N, 2, 128, NPIX
    ).transpose(0, 2, 1, 3)
    harr = harr.reshape(N, 128, 2 * H1BLK)

    return [
        {
            "xprep": np.ascontiguousarray(x_prep[c * BPC : (c + 1) * BPC]),
            "h1": np.ascontiguousarray(harr[c * BPC : (c + 1) * BPC]),
            "w3f": W3f, "w1f": W1f, "consts": consts,
        }
        for c in range(NCORES)
    ]


def finish_out(arr):
    """Per-core raw out [BPC,2,128,OP] bf16 -> [BPC,C,H,W] fp32 (+b23)."""
    out = np.asarray(arr).reshape(BPC, C, H, W).astype(np.float32)
    return out + _CACHE["b23"][None, :, None, None]


def assemble_out(results):
    outs = [finish_out(results[c]["out"]) for c in range(NCORES)]
    return np.ascontiguousarray(np.concatenate(outs, axis=0))


def _fallback_numpy(x, w3, w1, b11, b12, b13, b21, b22, b23,
                    g1, be1, m1, v1, g2, be2, m2, v2, a1, a2):
    # Straightforward reference math in numpy; only used if an assumption of
    # the device kernel (prelu slope a1 >= 0) is violated.
    def cb(p):
        return p[None, :, None, None]

    def conv_np(a, w, pad):
        N, Ci, Hh, Ww = a.shape
        O, I, kh, kw = w.shape
        ap = np.pad(a, ((0, 0), (0, 0), (pad, pad), (pad, pad)))
        out = np.zeros((N, O, Hh, Ww), np.float32)
        wm = w.reshape(O, -1)
        for n in range(N):
            cols = np.empty((I * kh * kw, Hh * Ww), np.float32)
            idx = 0
            for i in range(I):
                for dh in range(kh):
                    for dw in range(kw):
                        cols[idx] = ap[n, i, dh : dh + Hh, dw : dw + Ww].ravel()
                        idx += 1
            out[n] = (wm @ cols).reshape(O, Hh, Ww)
        return out

    def bn(t, g, b, mm, v):
        inv = g / np.sqrt(v + EPS)
        return t * cb(inv) + cb(b - mm * inv)

    def prelu(t, a):
        return np.where(t > 0, t, cb(a) * t)

    s3 = np.mean(np.abs(w3), axis=(1, 2, 3), keepdims=True)
    s1 = np.mean(np.abs(w1), axis=(1, 2, 3), keepdims=True)
    o1 = conv_np(np.sign(x + cb(b11)), np.sign(w3) * s3, 1)
    o1 = x + bn(o1, g1, be1, m1, v1)
    o1 = prelu(o1 + cb(b12), a1) + cb(b13)
    o2 = conv_np(np.sign(o1 + cb(b21)), np.sign(w1) * s1, 0)
    o2 = bn(o2, g2, be2, m2, v2) + o1
    o2 = prelu(o2 + cb(b22), a2) + cb(b23)
    return o2.astype(np.float32)


def kernel(**inputs):
    inputs = {k: np.asarray(v) for k, v in inputs.items()}
    if (np.asarray(inputs["a1"], np.float32) < 0).any():
        return _fallback_numpy(**{k: np.asarray(v, np.float32)
                                  for k, v in inputs.items()})
    in_maps = make_in_maps(**inputs)
    res = _run(in_maps, trace=False)
    return assemble_out(res.results)
